# revision 13
# baseline (speedup 1.0000x reference)
"""Trainium2 Bass kernel: batched complex-waveform similarity.

Math: reference computes
    bank = ifft_ortho(freq)                # [T, L] complex
    score = rx @ conj(bank).T              # [B, T] complex
    sim   = (score.re^2 + score.im^2) / temperature

Since the ortho DFT is unitary,  score = fft_ortho(rx) @ conj(freq).T.
So the kernel never builds the bank: it DFTs rx via a 128x128 bf16
matmul, then runs one big complex GEMM [B,L]x[L,T] in bf16 with fp32
PSUM accumulation, and a fused squared-magnitude epilogue.  The
1/temperature scale is folded into the DFT matrix host-side (score
scales by 1/sqrt(temp), sim by 1/temp), so the epilogue is exactly
sq(Sr) + Si^2 with no extra scale op.

Sharding: data-parallel over the rx batch dim across 8 NeuronCores;
freq (as a transposed bf16 [L, T] pair) is replicated on every core.

Per-core engine pipeline (main phase is PE-bound, ~216ns per matmul):
  PE   : DFT (bf16) + 512 bf16 matmuls [128,128]@[128,512] -> PSUM Sr/Si
  ACT  : t2 = Square(Si)                    (PSUM -> SBUF)
  DVE  : out = Sr^2 + t2                    (custom fused DVE op, 1cyc/elem)
  SP/ACT: HWDGE DMAs in/out on both rings
"""

import numpy as np
import ml_dtypes

B = 8192
T = 8192
L = 128
NCORES = 8
BPC = B // NCORES  # batch rows per core

_BF16 = ml_dtypes.bfloat16

_CACHE = {}


# --------------------------------------------------------------------------- #
# Custom DVE op: out = Src0^2 + Src1   (2 ALU ops -> 1 cycle/elem)
# (Src0 = Sr from PSUM, Src1 = Si^2 staged by ScalarE)
# --------------------------------------------------------------------------- #
def _get_sqadd_op():
    import concourse.dve_ops as dve_ops
    from concourse.dve_spec import Spec, Src0, Src1, sq, lower, _has_src1
    from concourse.dve_uop import DveOpSpec

    name = "SQ_ADD2_ANT"
    for op in dve_ops.OPS:
        if op.name == name:
            return op

    spec = Spec(
        body=sq(Src0) + Src1,
        reference=lambda in0, in1, s0, s1, imm2: (
            in0.astype(np.float32) ** 2 + in1.astype(np.float32)
        ).astype(np.float32),
    )
    opcode = dve_ops._CUSTOM_DVE_ROW_BASE + len(dve_ops.OPS)
    assert opcode < 0x20
    shas = {}
    for ver in ("v3", "v4"):
        compiled = DveOpSpec(
            name=name, opcode=opcode, uops=lower(spec, ver=ver), rd1_en=_has_src1(spec)
        )
        shas[ver] = compiled.sha(ver)
    op = dve_ops.DveOp(name, spec, subdim=False, uops_sha=shas)
    dve_ops.OPS.append(op)
    dve_ops.CUSTOM_DVE_SPECS[name] = spec
    dve_ops._SUB_OPCODE_FOR_NAME[name] = opcode
    return op


# --------------------------------------------------------------------------- #
# Bass program (one SPMD NeuronCore)
# --------------------------------------------------------------------------- #
def build_nc(bpc=BPC, t=T, debug=False):
    from contextlib import ExitStack

    import concourse.bacc as bacc
    import concourse.bass as bass
    import concourse.mybir as mybir
    import concourse.tile as tile

    f32 = mybir.dt.float32
    bf16 = mybir.dt.bfloat16
    sqadd = _get_sqadd_op()

    NG = 512   # output columns per PSUM group (1 bank)
    FG = 1024  # freq columns per SBUF tile / DMA
    OBW = 2048  # out staging tile width: 4 groups, 8KB rows per DMA
    assert bpc % 128 == 0 and t % FG == 0

    nc = bacc.Bacc("TRN2", target_bir_lowering=False, debug=debug, num_devices=NCORES)

    # packed inputs: 4KB DRAM rows -> full-size DMA packets
    rxp = nc.dram_tensor("rxp", [L, 2 * bpc], bf16, kind="ExternalInput")
    fqp = nc.dram_tensor("fqp", [L, 2 * t], bf16, kind="ExternalInput")
    wp = nc.dram_tensor("wp", [L, 3 * L], bf16, kind="ExternalInput")
    out = nc.dram_tensor("out", [bpc, t], f32, kind="ExternalOutput")

    with tile.TileContext(nc) as tc, ExitStack() as ctx:
        consts = ctx.enter_context(tc.tile_pool(name="consts", bufs=1))
        psum = ctx.enter_context(
            tc.tile_pool(name="psum", bufs=4, space=bass.MemorySpace.PSUM)
        )
        sq_pool = ctx.enter_context(tc.tile_pool(name="sq", bufs=6))
        out_pool = ctx.enter_context(tc.tile_pool(name="ob", bufs=4))

        # ---- input DMA triggers first ---------------------------------- #
        # Everything packed to 4KB DRAM rows.  rx (one 512KB DMA) on the SP
        # ring; W + freq group 0 lead the ScalarE ring; remaining freq
        # groups (512KB [fr_g|fi_g] pairs) alternate across both rings.
        # rx in 4 chunked DMAs alternating rings: chunk c holds
        # [rxr[:, c*256:(c+1)*256] | rxi[:, ...]] so DFT chunk c waits only
        # on its own 128KB transfer, not the whole 512KB.
        w_sb = consts.tile([L, 3 * L], bf16)
        nc.scalar.dma_start(w_sb[:], wp[:, :])  # tiny, must precede rx on ring
        RC = 256  # rx cols per chunk (per r/i half)
        nrc = bpc // RC
        rx_ck = []
        for c in range(nrc):
            rc = consts.tile([L, 2 * RC], bf16, tag=f"rx{c}")
            eng = nc.sync if c % 2 == 0 else nc.scalar
            eng.dma_start(rc[:], rxp[:, c * 2 * RC : (c + 1) * 2 * RC])
            rx_ck.append(rc)
        fq_sb = []
        for g in range(t // FG):
            gs = slice(g * 2 * FG, (g + 1) * 2 * FG)
            fq = consts.tile([L, 2 * FG], bf16, tag=f"fq{g}")
            eng = nc.scalar if g % 2 == 0 else nc.sync
            eng.dma_start(fq[:], fqp[:, gs])
            fq_sb.append(fq)

        # ---- PE warmup -------------------------------------------------- #
        # Dependency-free matmuls ramp the HAM clock gate while rx loads.
        warm_w = consts.tile([128, 128], bf16)
        nc.gpsimd.memset(warm_w[:], 0)
        warm_ps = psum.tile([128, NG], mybir.dt.float32, tag="si")
        for _ in range(24):
            nc.tensor.matmul(warm_ps[:, 0:128], warm_w[:], warm_w[:], start=True, stop=True)

        # ---- DFT of rx (bf16): rxfT = W' @ rxT -------------------------- #
        # W' = ortho DFT matrix / sqrt(temp), symmetric, so PE lhsT is W'.
        # rxfT_r = Wr@rxT_r - Wi@rxT_i ; rxfT_i = Wr@rxT_i + Wi@rxT_r
        rxf_r = consts.tile([L, bpc], bf16)
        rxf_i = consts.tile([L, bpc], bf16)
        rxf_nr = consts.tile([L, bpc], bf16)  # -rxfT_r
        wr = slice(0, L)
        wni = slice(L, 2 * L)
        wi = slice(2 * L, 3 * L)
        for c in range(nrc):
            rc = rx_ck[c]
            crr = slice(0, RC)        # rx_real cols in chunk
            cii = slice(RC, 2 * RC)   # rx_imag cols in chunk
            ks = slice(c * RC, (c + 1) * RC)  # dest cols in rxf
            pr = psum.tile([128, NG], mybir.dt.float32, tag="sr")
            nc.tensor.matmul(pr[:, 0:RC], w_sb[:, wr], rc[:, crr], start=True, stop=False)
            nc.tensor.matmul(pr[:, 0:RC], w_sb[:, wni], rc[:, cii], start=False, stop=True)
            pi = psum.tile([128, NG], mybir.dt.float32, tag="si")
            nc.tensor.matmul(pi[:, 0:RC], w_sb[:, wr], rc[:, cii], start=True, stop=False)
            nc.tensor.matmul(pi[:, 0:RC], w_sb[:, wi], rc[:, crr], start=False, stop=True)
            nc.vector.tensor_copy(rxf_r[:, ks], pr[:, 0:RC])
            nc.vector.tensor_copy(rxf_i[:, ks], pi[:, 0:RC])
            nc.vector.tensor_scalar_mul(rxf_nr[:, ks], pr[:, 0:RC], -1.0)

        # ---- main complex GEMM + fused |.|^2 epilogue ------------------- #
        # Sr = rxf_r.T @ fr + rxf_i.T @ fi
        # Si = rxf_i.T @ fr - rxf_r.T @ fi
        for m in range(bpc // 128):
            ms = slice(m * 128, (m + 1) * 128)
            last_m = m == bpc // 128 - 1
            ob = None
            for n in range(t // NG):
                g, j = divmod(n, FG // NG)
                jr = slice(j * NG, (j + 1) * NG)           # fr cols in fq tile
                ji = slice(FG + j * NG, FG + (j + 1) * NG)  # fi cols in fq tile
                fq = fq_sb[g]
                sr = psum.tile([128, NG], mybir.dt.float32, tag="sr")
                si = psum.tile([128, NG], mybir.dt.float32, tag="si")
                nc.tensor.matmul(sr[:], rxf_r[:, ms], fq[:, jr], start=True, stop=False)
                nc.tensor.matmul(sr[:], rxf_i[:, ms], fq[:, ji], start=False, stop=True)
                nc.tensor.matmul(si[:], rxf_i[:, ms], fq[:, jr], start=True, stop=False)
                nc.tensor.matmul(si[:], rxf_nr[:, ms], fq[:, ji], start=False, stop=True)
                t2 = sq_pool.tile([128, NG], f32)
                nc.scalar.square(t2[:], si[:])
                o = n % (OBW // NG)
                if o == 0:
                    ob = out_pool.tile([128, OBW], f32)
                nc.vector._custom_dve(
                    sqadd,
                    out=ob[:, o * NG : (o + 1) * NG],
                    in0=sr[:],
                    in1=t2[:],
                )
                if last_m:
                    # final row: per-group DMAs on alternating rings so the
                    # kernel-exit barrier waits on small transfers only
                    if n == t // NG - 1:
                        # very last group: split across both rings
                        h = NG // 2
                        nc.sync.dma_start(
                            out[ms, n * NG : n * NG + h], ob[:, o * NG : o * NG + h]
                        )
                        nc.scalar.dma_start(
                            out[ms, n * NG + h : (n + 1) * NG],
                            ob[:, o * NG + h : (o + 1) * NG],
                        )
                    else:
                        oeng = nc.sync if n % 2 == 0 else nc.scalar
                        oeng.dma_start(
                            out[ms, n * NG : (n + 1) * NG], ob[:, o * NG : (o + 1) * NG]
                        )
                elif o == OBW // NG - 1:
                    n0 = n - o
                    # alternate big output DMAs across both HWDGE rings
                    pair = (m * (t // NG) + n) // (OBW // NG)
                    oeng = nc.scalar if pair % 2 == 0 else nc.sync
                    oeng.dma_start(out[ms, n0 * NG : n0 * NG + OBW], ob[:])

    nc.compile()
    return nc


def _host_prep(rx_real, rx_imag, freq_real, freq_imag, temperature, bpc=BPC, t=T):
    """Layout marshaling only: shard/transpose/cast inputs for the cores."""
    FG = 1024
    lk = np.outer(np.arange(L), np.arange(L)).astype(np.float64)
    w = np.exp(-2j * np.pi * lk / L) / np.sqrt(L)  # ortho DFT matrix (symmetric)
    # fold the temperature scale into the DFT matrix: sim scales by 1/temp
    w = w / np.sqrt(np.float64(np.asarray(temperature)))
    w_r = w.real.astype(np.float32).astype(_BF16)
    w_i = w.imag.astype(np.float32).astype(_BF16)
    # packed [wr | -wi | wi], 4KB-class rows
    wp = np.ascontiguousarray(np.concatenate([w_r, -w_i, w_i], axis=1))

    fqt_r = freq_real[:t].T.astype(_BF16)  # [L, T]
    fqt_i = freq_imag[:t].T.astype(_BF16)
    # packed freq: per group g of FG columns, [fr_g | fi_g] -> 4KB rows
    fqp = np.empty((L, 2 * t), _BF16)
    for g in range(t // FG):
        fqp[:, 2 * g * FG : (2 * g + 1) * FG] = fqt_r[:, g * FG : (g + 1) * FG]
        fqp[:, (2 * g + 1) * FG : (2 * g + 2) * FG] = fqt_i[:, g * FG : (g + 1) * FG]
    fqp = np.ascontiguousarray(fqp)

    rxt_r = np.asarray(rx_real, np.float32).T.astype(_BF16)  # [L, B]
    rxt_i = np.asarray(rx_imag, np.float32).T.astype(_BF16)

    RC = 256  # must match kernel: per-chunk [rxr_c | rxi_c]
    in_maps = []
    for c in range(NCORES):
        cs = slice(c * bpc, (c + 1) * bpc)
        rr, ri = rxt_r[:, cs], rxt_i[:, cs]
        rxp = np.empty((L, 2 * bpc), _BF16)
        for k in range(bpc // RC):
            rxp[:, 2 * k * RC : (2 * k + 1) * RC] = rr[:, k * RC : (k + 1) * RC]
            rxp[:, (2 * k + 1) * RC : (2 * k + 2) * RC] = ri[:, k * RC : (k + 1) * RC]
        in_maps.append({"rxp": np.ascontiguousarray(rxp), "fqp": fqp, "wp": wp})
    return in_maps


def kernel(rx_real, rx_imag, freq_real, freq_imag, temperature):
    from concourse.bass_utils import run_bass_kernel_spmd

    if "nc" not in _CACHE:
        _CACHE["nc"] = build_nc()
    nc = _CACHE["nc"]

    in_maps = _host_prep(rx_real, rx_imag, freq_real, freq_imag, temperature)
    res = run_bass_kernel_spmd(nc, in_maps, core_ids=list(range(NCORES)))
    _CACHE["last_result"] = res
    return np.concatenate([r["out"] for r in res.results], axis=0)


# revision 22
# speedup vs baseline: 1.0016x; 1.0016x over previous
"""Trainium2 Bass kernel: batched complex-waveform similarity.

Math: reference computes
    bank = ifft_ortho(freq)                # [T, L] complex
    score = rx @ conj(bank).T              # [B, T] complex
    sim   = (score.re^2 + score.im^2) / temperature

Since the ortho DFT is unitary,  score = fft_ortho(rx) @ conj(freq).T.
So the kernel never builds the bank: it DFTs rx via a 128x128 bf16
matmul, then runs one big complex GEMM [B,L]x[L,T] in bf16 with fp32
PSUM accumulation, and a fused squared-magnitude epilogue.  The
1/temperature scale is folded into the DFT matrix host-side (score
scales by 1/sqrt(temp), sim by 1/temp), so the epilogue is exactly
sq(Sr) + Si^2 with no extra scale op.

Sharding: data-parallel over the rx batch dim across 8 NeuronCores;
freq (as a transposed bf16 [L, T] pair) is replicated on every core.

Per-core engine pipeline (main phase is PE-bound, ~216ns per matmul):
  PE   : DFT (bf16) + 512 bf16 matmuls [128,128]@[128,512] -> PSUM Sr/Si
  ACT  : t2 = Square(Si)                    (PSUM -> SBUF)
  DVE  : out = Sr^2 + t2                    (custom fused DVE op, 1cyc/elem)
  SP/ACT: HWDGE DMAs in/out on both rings
"""

import numpy as np
import ml_dtypes

B = 8192
T = 8192
L = 128
NCORES = 8
BPC = B // NCORES  # batch rows per core

_BF16 = ml_dtypes.bfloat16

_CACHE = {}


# --------------------------------------------------------------------------- #
# Custom DVE op: out = Src0^2 + Src1   (2 ALU ops -> 1 cycle/elem)
# (Src0 = Sr from PSUM, Src1 = Si^2 staged by ScalarE)
# --------------------------------------------------------------------------- #
def _get_sqadd_op():
    import concourse.dve_ops as dve_ops
    from concourse.dve_spec import Spec, Src0, Src1, sq, lower, _has_src1
    from concourse.dve_uop import DveOpSpec

    name = "SQ_ADD2_ANT"
    for op in dve_ops.OPS:
        if op.name == name:
            return op

    spec = Spec(
        body=sq(Src0) + Src1,
        reference=lambda in0, in1, s0, s1, imm2: (
            in0.astype(np.float32) ** 2 + in1.astype(np.float32)
        ).astype(np.float32),
    )
    opcode = dve_ops._CUSTOM_DVE_ROW_BASE + len(dve_ops.OPS)
    assert opcode < 0x20
    shas = {}
    for ver in ("v3", "v4"):
        compiled = DveOpSpec(
            name=name, opcode=opcode, uops=lower(spec, ver=ver), rd1_en=_has_src1(spec)
        )
        shas[ver] = compiled.sha(ver)
    op = dve_ops.DveOp(name, spec, subdim=False, uops_sha=shas)
    dve_ops.OPS.append(op)
    dve_ops.CUSTOM_DVE_SPECS[name] = spec
    dve_ops._SUB_OPCODE_FOR_NAME[name] = opcode
    return op


# --------------------------------------------------------------------------- #
# Bass program (one SPMD NeuronCore)
# --------------------------------------------------------------------------- #
def build_nc(bpc=BPC, t=T, debug=False):
    from contextlib import ExitStack

    import concourse.bacc as bacc
    import concourse.bass as bass
    import concourse.mybir as mybir
    import concourse.tile as tile

    f32 = mybir.dt.float32
    bf16 = mybir.dt.bfloat16
    sqadd = _get_sqadd_op()

    NG = 512   # output columns per PSUM group (1 bank)
    FG = 1024  # freq columns per SBUF tile / DMA
    assert bpc % 128 == 0 and t % FG == 0

    nc = bacc.Bacc("TRN2", target_bir_lowering=False, debug=debug, num_devices=NCORES)

    # packed inputs: 4KB DRAM rows -> full-size DMA packets
    rxp = nc.dram_tensor("rxp", [L, 2 * bpc], bf16, kind="ExternalInput")
    fqp = nc.dram_tensor("fqp", [L, 2 * t], bf16, kind="ExternalInput")
    wp = nc.dram_tensor("wp", [L, 3 * L], bf16, kind="ExternalInput")
    out = nc.dram_tensor("out", [bpc, t], f32, kind="ExternalOutput")

    with tile.TileContext(nc) as tc, ExitStack() as ctx:
        consts = ctx.enter_context(tc.tile_pool(name="consts", bufs=1))
        psum = ctx.enter_context(
            tc.tile_pool(name="psum", bufs=4, space=bass.MemorySpace.PSUM)
        )
        sq_pool = ctx.enter_context(tc.tile_pool(name="sq", bufs=6))
        out_pool = ctx.enter_context(tc.tile_pool(name="ob", bufs=8))

        # ---- input DMA triggers first ---------------------------------- #
        # Everything packed to 4KB DRAM rows.  rx (one 512KB DMA) on the SP
        # ring; W + freq group 0 lead the ScalarE ring; remaining freq
        # groups (512KB [fr_g|fi_g] pairs) alternate across both rings.
        # rx in 4 chunked DMAs alternating rings: chunk c holds
        # [rxr[:, c*256:(c+1)*256] | rxi[:, ...]] so DFT chunk c waits only
        # on its own 128KB transfer, not the whole 512KB.
        # Ring schedule (the first ~10us of DMA runs at ramp-limited rate, so
        # order by when the g-major main loop needs each piece):
        #   SP ring    : rx_c0, rx_c2, fq1, fq2, fq4, fq6
        #   ScalarE    : wp, fq0a, fq0b, rx_c1, rx_c3, fq3, fq5, fq7
        RC = 256  # rx cols per chunk (per r/i half)
        nrc = bpc // RC
        w_sb = consts.tile([L, 3 * L], bf16)
        nc.scalar.dma_start(w_sb[:], wp[:, :])
        rx_ck = [
            consts.tile([L, 2 * RC], bf16, tag=f"rx{c}", name=f"rx{c}")
            for c in range(nrc)
        ]
        nc.sync.dma_start(rx_ck[0][:], rxp[:, 0 : 2 * RC])
        fq_sb = [
            consts.tile([L, 2 * FG], bf16, tag=f"fq{g}", name=f"fq{g}")
            for g in range(t // FG)
        ]
        # group 0 arrives as two quarter-interleaved halves (see _host_prep)
        nc.scalar.dma_start(fq_sb[0][:, 0:FG], fqp[:, 0:FG])
        nc.sync.dma_start(rx_ck[2][:], rxp[:, 4 * RC : 6 * RC])
        nc.scalar.dma_start(fq_sb[0][:, FG : 2 * FG], fqp[:, FG : 2 * FG])
        nc.scalar.dma_start(rx_ck[1][:], rxp[:, 2 * RC : 4 * RC])
        nc.scalar.dma_start(rx_ck[3][:], rxp[:, 6 * RC : 8 * RC])
        for g in range(1, t // FG):
            gs = slice(g * 2 * FG, (g + 1) * 2 * FG)
            eng = nc.sync if g % 2 == 1 else nc.scalar
            eng.dma_start(fq_sb[g][:], fqp[:, gs])

        # ---- PE warmup -------------------------------------------------- #
        # Dependency-free matmuls ramp the HAM clock gate while rx loads.
        warm_w = consts.tile([128, 128], bf16)
        nc.gpsimd.memset(warm_w[:], 0)
        warm_ps = psum.tile([128, NG], mybir.dt.float32, tag="si")
        for _ in range(24):
            nc.tensor.matmul(warm_ps[:, 0:128], warm_w[:], warm_w[:], start=True, stop=True)

        # ---- DFT of rx (bf16): rxfT = W' @ rxT -------------------------- #
        # W' = ortho DFT matrix / sqrt(temp), symmetric, so PE lhsT is W'.
        # rxfT_r = Wr@rxT_r - Wi@rxT_i ; rxfT_i = Wr@rxT_i + Wi@rxT_r
        rxf_r = consts.tile([L, bpc], bf16)
        rxf_i = consts.tile([L, bpc], bf16)
        rxf_nr = consts.tile([L, bpc], bf16)  # -rxfT_r
        wr = slice(0, L)
        wni = slice(L, 2 * L)
        wi = slice(2 * L, 3 * L)
        MB = bpc // 128
        NGB = t // FG  # fq tiles; each covers 2 NG-wide output groups
        f32p = mybir.dt.float32

        def dft_chunk(c):
            # rxfT chunk c (rxf cols [c*RC,(c+1)*RC] = m-blocks 2c, 2c+1)
            rc = rx_ck[c]
            ks = slice(c * RC, (c + 1) * RC)
            pr = psum.tile([128, NG], f32p, tag="sr")
            nc.tensor.matmul(pr[:, 0:RC], w_sb[:, wr], rc[:, 0:RC], start=True, stop=False)
            nc.tensor.matmul(pr[:, 0:RC], w_sb[:, wni], rc[:, RC : 2 * RC], start=False, stop=True)
            pi = psum.tile([128, NG], f32p, tag="si")
            nc.tensor.matmul(pi[:, 0:RC], w_sb[:, wr], rc[:, RC : 2 * RC], start=True, stop=False)
            nc.tensor.matmul(pi[:, 0:RC], w_sb[:, wi], rc[:, 0:RC], start=False, stop=True)
            nc.vector.tensor_copy(rxf_r[:, ks], pr[:, 0:RC])
            nc.vector.tensor_copy(rxf_i[:, ks], pi[:, 0:RC])
            nc.vector.tensor_scalar_mul(rxf_nr[:, ks], pr[:, 0:RC], -1.0)

        # ---- main complex GEMM + fused |.|^2 epilogue ------------------- #
        # Sr = rxf_r.T @ fr + rxf_i.T @ fi ; Si = rxf_i.T @ fr - rxf_r.T @ fi
        # g-major order: each 512KB freq pair feeds 13.8us of PE work, so
        # the ramp-limited early DMA never stalls the PE after tile 0.
        def tile_mg(m, gb, j, ob):
            ms = slice(m * 128, (m + 1) * 128)
            fq = fq_sb[gb]
            if gb == 0:
                # group 0 is quarter-interleaved [fr00|fi00|fr01|fi01]
                jr = slice(j * 2 * NG, j * 2 * NG + NG)
                ji = slice(j * 2 * NG + NG, (j + 1) * 2 * NG)
            else:
                jr = slice(j * NG, (j + 1) * NG)
                ji = slice(FG + j * NG, FG + (j + 1) * NG)
            sr = psum.tile([128, NG], f32p, tag="sr")
            si = psum.tile([128, NG], f32p, tag="si")
            # si first: the ACT square overlaps the sr matmuls, keeping it
            # off the critical path after the tile's last matmul
            nc.tensor.matmul(si[:], rxf_i[:, ms], fq[:, jr], start=True, stop=False)
            nc.tensor.matmul(si[:], rxf_nr[:, ms], fq[:, ji], start=False, stop=True)
            nc.tensor.matmul(sr[:], rxf_r[:, ms], fq[:, jr], start=True, stop=False)
            nc.tensor.matmul(sr[:], rxf_i[:, ms], fq[:, ji], start=False, stop=True)
            t2 = sq_pool.tile([128, NG], f32)
            nc.scalar.square(t2[:], si[:])
            nc.vector._custom_dve(
                sqadd, out=ob[:, j * NG : (j + 1) * NG], in0=sr[:], in1=t2[:]
            )

        def emit_out(m, gb, ob):
            ms = slice(m * 128, (m + 1) * 128)
            c0 = gb * 2 * NG
            if m == MB - 1 and gb == NGB - 1:
                # very last tile: split across both rings for a short drain
                nc.sync.dma_start(out[ms, c0 : c0 + NG], ob[:, 0:NG])
                nc.scalar.dma_start(out[ms, c0 + NG : c0 + 2 * NG], ob[:, NG : 2 * NG])
            else:
                eng = nc.sync if (gb * MB + m) % 2 == 0 else nc.scalar
                eng.dma_start(out[ms, c0 : c0 + 2 * NG], ob[:])

        # gb=0 with the DFT interleaved: chunk c unlocks m-blocks 2c, 2c+1
        obs = {}
        dft_chunk(0)
        for ci in range(nrc):
            ma, mb_ = 2 * ci, 2 * ci + 1
            for m in (ma, mb_):
                obs[m] = out_pool.tile([128, 2 * NG], f32, name=f"ob0_{m}", tag="ob")
                tile_mg(m, 0, 0, obs[m])
            for m in (ma, mb_):
                tile_mg(m, 0, 1, obs[m])
                emit_out(m, 0, obs[m])
            if ci + 1 < nrc:
                dft_chunk(ci + 1)

        for gb in range(1, NGB):
            for m in range(MB):
                ob = out_pool.tile([128, 2 * NG], f32, tag="ob")
                tile_mg(m, gb, 0, ob)
                tile_mg(m, gb, 1, ob)
                emit_out(m, gb, ob)

    nc.compile()
    return nc


def _host_prep(rx_real, rx_imag, freq_real, freq_imag, temperature, bpc=BPC, t=T):
    """Layout marshaling only: shard/transpose/cast inputs for the cores."""
    FG = 1024
    lk = np.outer(np.arange(L), np.arange(L)).astype(np.float64)
    w = np.exp(-2j * np.pi * lk / L) / np.sqrt(L)  # ortho DFT matrix (symmetric)
    # fold the temperature scale into the DFT matrix: sim scales by 1/temp
    w = w / np.sqrt(np.float64(np.asarray(temperature)))
    w_r = w.real.astype(np.float32).astype(_BF16)
    w_i = w.imag.astype(np.float32).astype(_BF16)
    # packed [wr | -wi | wi], 4KB-class rows
    wp = np.ascontiguousarray(np.concatenate([w_r, -w_i, w_i], axis=1))

    fqt_r = freq_real[:t].T.astype(_BF16)  # [L, T]
    fqt_i = freq_imag[:t].T.astype(_BF16)
    # packed freq: per group g of FG columns, [fr_g | fi_g] -> 4KB rows.
    # group 0 is quarter-interleaved [fr00|fi00|fr01|fi01] so it can ship
    # as two half-size DMAs that unlock the first output tiles sooner.
    fqp = np.empty((L, 2 * t), _BF16)
    for g in range(t // FG):
        fqp[:, 2 * g * FG : (2 * g + 1) * FG] = fqt_r[:, g * FG : (g + 1) * FG]
        fqp[:, (2 * g + 1) * FG : (2 * g + 2) * FG] = fqt_i[:, g * FG : (g + 1) * FG]
    h = FG // 2
    g0 = np.concatenate(
        [fqt_r[:, 0:h], fqt_i[:, 0:h], fqt_r[:, h:FG], fqt_i[:, h:FG]], axis=1
    )
    fqp[:, 0 : 2 * FG] = g0
    fqp = np.ascontiguousarray(fqp)

    rxt_r = np.asarray(rx_real, np.float32).T.astype(_BF16)  # [L, B]
    rxt_i = np.asarray(rx_imag, np.float32).T.astype(_BF16)

    RC = 256  # must match kernel: per-chunk [rxr_c | rxi_c]
    in_maps = []
    for c in range(NCORES):
        cs = slice(c * bpc, (c + 1) * bpc)
        rr, ri = rxt_r[:, cs], rxt_i[:, cs]
        rxp = np.empty((L, 2 * bpc), _BF16)
        for k in range(bpc // RC):
            rxp[:, 2 * k * RC : (2 * k + 1) * RC] = rr[:, k * RC : (k + 1) * RC]
            rxp[:, (2 * k + 1) * RC : (2 * k + 2) * RC] = ri[:, k * RC : (k + 1) * RC]
        in_maps.append({"rxp": np.ascontiguousarray(rxp), "fqp": fqp, "wp": wp})
    return in_maps


def kernel(rx_real, rx_imag, freq_real, freq_imag, temperature):
    from concourse.bass_utils import run_bass_kernel_spmd

    if "nc" not in _CACHE:
        _CACHE["nc"] = build_nc()
    nc = _CACHE["nc"]

    in_maps = _host_prep(rx_real, rx_imag, freq_real, freq_imag, temperature)
    res = run_bass_kernel_spmd(nc, in_maps, core_ids=list(range(NCORES)))
    _CACHE["last_result"] = res
    return np.concatenate([r["out"] for r in res.results], axis=0)


# revision 24
# speedup vs baseline: 1.0147x; 1.0131x over previous
"""Trainium2 Bass kernel: batched complex-waveform similarity.

Math: reference computes
    bank = ifft_ortho(freq)                # [T, L] complex
    score = rx @ conj(bank).T              # [B, T] complex
    sim   = (score.re^2 + score.im^2) / temperature

Since the ortho DFT is unitary,  score = fft_ortho(rx) @ conj(freq).T.
So the kernel never builds the bank: it DFTs rx via a 128x128 bf16
matmul, then runs one big complex GEMM [B,L]x[L,T] in bf16 with fp32
PSUM accumulation, and a fused squared-magnitude epilogue.  The
1/temperature scale is folded into the DFT matrix host-side (score
scales by 1/sqrt(temp), sim by 1/temp), so the epilogue is exactly
sq(Sr) + Si^2 with no extra scale op.

Sharding: data-parallel over the rx batch dim across 8 NeuronCores;
freq (as a transposed bf16 [L, T] pair) is replicated on every core.

Per-core engine pipeline (main phase is PE-bound, ~216ns per matmul):
  PE   : DFT (bf16) + 512 bf16 matmuls [128,128]@[128,512] -> PSUM Sr/Si
  ACT  : t2 = Square(Si)                    (PSUM -> SBUF)
  DVE  : out = Sr^2 + t2                    (custom fused DVE op, 1cyc/elem)
  SP/ACT: HWDGE DMAs in/out on both rings
"""

import numpy as np
import ml_dtypes

B = 8192
T = 8192
L = 128
NCORES = 8
BPC = B // NCORES  # batch rows per core

_BF16 = ml_dtypes.bfloat16

_CACHE = {}


# --------------------------------------------------------------------------- #
# Custom DVE op: out = Src0^2 + Src1   (2 ALU ops -> 1 cycle/elem)
# (Src0 = Sr from PSUM, Src1 = Si^2 staged by ScalarE)
# --------------------------------------------------------------------------- #
def _get_sqadd_op():
    import concourse.dve_ops as dve_ops
    from concourse.dve_spec import Spec, Src0, Src1, sq, lower, _has_src1
    from concourse.dve_uop import DveOpSpec

    name = "SQ_ADD2_ANT"
    for op in dve_ops.OPS:
        if op.name == name:
            return op

    spec = Spec(
        body=sq(Src0) + Src1,
        reference=lambda in0, in1, s0, s1, imm2: (
            in0.astype(np.float32) ** 2 + in1.astype(np.float32)
        ).astype(np.float32),
    )
    opcode = dve_ops._CUSTOM_DVE_ROW_BASE + len(dve_ops.OPS)
    assert opcode < 0x20
    shas = {}
    for ver in ("v3", "v4"):
        compiled = DveOpSpec(
            name=name, opcode=opcode, uops=lower(spec, ver=ver), rd1_en=_has_src1(spec)
        )
        shas[ver] = compiled.sha(ver)
    op = dve_ops.DveOp(name, spec, subdim=False, uops_sha=shas)
    dve_ops.OPS.append(op)
    dve_ops.CUSTOM_DVE_SPECS[name] = spec
    dve_ops._SUB_OPCODE_FOR_NAME[name] = opcode
    return op


# --------------------------------------------------------------------------- #
# Bass program (one SPMD NeuronCore)
# --------------------------------------------------------------------------- #
def build_nc(bpc=BPC, t=T, debug=False):
    from contextlib import ExitStack

    import concourse.bacc as bacc
    import concourse.bass as bass
    import concourse.mybir as mybir
    import concourse.tile as tile

    f32 = mybir.dt.float32
    bf16 = mybir.dt.bfloat16
    sqadd = _get_sqadd_op()

    NG = 512   # output columns per PSUM group (1 bank)
    FG = 1024  # freq columns per SBUF tile / DMA
    assert bpc % 128 == 0 and t % FG == 0

    nc = bacc.Bacc("TRN2", target_bir_lowering=False, debug=debug, num_devices=NCORES)

    # packed inputs: 4KB DRAM rows -> full-size DMA packets
    rxp = nc.dram_tensor("rxp", [L, 2 * bpc], bf16, kind="ExternalInput")
    fqp = nc.dram_tensor("fqp", [L, 2 * t], bf16, kind="ExternalInput")
    wp = nc.dram_tensor("wp", [L, 3 * L], bf16, kind="ExternalInput")
    out = nc.dram_tensor("out", [bpc, t], f32, kind="ExternalOutput")

    with tile.TileContext(nc) as tc, ExitStack() as ctx:
        consts = ctx.enter_context(tc.tile_pool(name="consts", bufs=1))
        psum = ctx.enter_context(
            tc.tile_pool(name="psum", bufs=4, space=bass.MemorySpace.PSUM)
        )
        sq_pool = ctx.enter_context(tc.tile_pool(name="sq", bufs=6))
        out_pool = ctx.enter_context(tc.tile_pool(name="ob", bufs=14))

        # ---- input DMA triggers first ---------------------------------- #
        # Everything packed to 4KB DRAM rows.  rx (one 512KB DMA) on the SP
        # ring; W + freq group 0 lead the ScalarE ring; remaining freq
        # groups (512KB [fr_g|fi_g] pairs) alternate across both rings.
        # rx in 4 chunked DMAs alternating rings: chunk c holds
        # [rxr[:, c*256:(c+1)*256] | rxi[:, ...]] so DFT chunk c waits only
        # on its own 128KB transfer, not the whole 512KB.
        # Ring schedule (the first ~10us of DMA runs at ramp-limited rate, so
        # order by when the g-major main loop needs each piece):
        #   SP ring    : rx_c0, rx_c2, fq1, fq2, fq4, fq6
        #   ScalarE    : wp, fq0a, fq0b, rx_c1, rx_c3, fq3, fq5, fq7
        RC = 256  # rx cols per chunk (per r/i half)
        nrc = bpc // RC
        w_sb = consts.tile([L, 3 * L], bf16)
        nc.sync.dma_start(w_sb[:], wp[:, :])
        rx_ck = [
            consts.tile([L, 2 * RC], bf16, tag=f"rx{c}", name=f"rx{c}")
            for c in range(nrc)
        ]
        fq_sb = [
            consts.tile([L, 2 * FG], bf16, tag=f"fq{g}", name=f"fq{g}")
            for g in range(t // FG)
        ]
        # group 0 leads the ScalarE ring as two quarter-interleaved halves
        # (see _host_prep); wp + all rx chunks stream on the SP ring in
        # parallel, so tile(0,0)'s inputs race through both rings at once.
        nc.scalar.dma_start(fq_sb[0][:, 0:FG], fqp[:, 0:FG])
        for c in range(nrc):
            nc.sync.dma_start(rx_ck[c][:], rxp[:, c * 2 * RC : (c + 1) * 2 * RC])
        nc.scalar.dma_start(fq_sb[0][:, FG : 2 * FG], fqp[:, FG : 2 * FG])
        # remaining groups: deadline for fq_g is ~13.8us*g after main start,
        # so ring balance barely matters; odd->SP (after rx), even->ScalarE.
        for g in range(1, t // FG):
            gs = slice(g * 2 * FG, (g + 1) * 2 * FG)
            eng = nc.sync if g % 2 == 1 else nc.scalar
            eng.dma_start(fq_sb[g][:], fqp[:, gs])

        # ---- PE warmup -------------------------------------------------- #
        # Dependency-free matmuls ramp the HAM clock gate while rx loads.
        warm_w = consts.tile([128, 128], bf16)
        nc.gpsimd.memset(warm_w[:], 0)
        warm_ps = psum.tile([128, NG], mybir.dt.float32, tag="si")
        for _ in range(24):
            nc.tensor.matmul(warm_ps[:, 0:128], warm_w[:], warm_w[:], start=True, stop=True)

        # ---- DFT of rx (bf16): rxfT = W' @ rxT -------------------------- #
        # W' = ortho DFT matrix / sqrt(temp), symmetric, so PE lhsT is W'.
        # rxfT_r = Wr@rxT_r - Wi@rxT_i ; rxfT_i = Wr@rxT_i + Wi@rxT_r
        rxf_r = consts.tile([L, bpc], bf16)
        rxf_i = consts.tile([L, bpc], bf16)
        rxf_nr = consts.tile([L, bpc], bf16)  # -rxfT_r
        wr = slice(0, L)
        wni = slice(L, 2 * L)
        wi = slice(2 * L, 3 * L)
        MB = bpc // 128
        NGB = t // FG  # fq tiles; each covers 2 NG-wide output groups
        f32p = mybir.dt.float32

        def dft_chunk(c):
            # rxfT chunk c (rxf cols [c*RC,(c+1)*RC] = m-blocks 2c, 2c+1)
            rc = rx_ck[c]
            ks = slice(c * RC, (c + 1) * RC)
            pr = psum.tile([128, NG], f32p, tag="sr")
            nc.tensor.matmul(pr[:, 0:RC], w_sb[:, wr], rc[:, 0:RC], start=True, stop=False)
            nc.tensor.matmul(pr[:, 0:RC], w_sb[:, wni], rc[:, RC : 2 * RC], start=False, stop=True)
            pi = psum.tile([128, NG], f32p, tag="si")
            nc.tensor.matmul(pi[:, 0:RC], w_sb[:, wr], rc[:, RC : 2 * RC], start=True, stop=False)
            nc.tensor.matmul(pi[:, 0:RC], w_sb[:, wi], rc[:, 0:RC], start=False, stop=True)
            nc.vector.tensor_copy(rxf_r[:, ks], pr[:, 0:RC])
            nc.vector.tensor_copy(rxf_i[:, ks], pi[:, 0:RC])
            nc.vector.tensor_scalar_mul(rxf_nr[:, ks], pr[:, 0:RC], -1.0)

        # ---- main complex GEMM + fused |.|^2 epilogue ------------------- #
        # Sr = rxf_r.T @ fr + rxf_i.T @ fi ; Si = rxf_i.T @ fr - rxf_r.T @ fi
        # g-major order: each 512KB freq pair feeds 13.8us of PE work, so
        # the ramp-limited early DMA never stalls the PE after tile 0.
        def tile_mg(m, gb, j, ob):
            ms = slice(m * 128, (m + 1) * 128)
            fq = fq_sb[gb]
            if gb == 0:
                # group 0 is quarter-interleaved [fr00|fi00|fr01|fi01]
                jr = slice(j * 2 * NG, j * 2 * NG + NG)
                ji = slice(j * 2 * NG + NG, (j + 1) * 2 * NG)
            else:
                jr = slice(j * NG, (j + 1) * NG)
                ji = slice(FG + j * NG, FG + (j + 1) * NG)
            sr = psum.tile([128, NG], f32p, tag="sr")
            si = psum.tile([128, NG], f32p, tag="si")
            # si first: the ACT square overlaps the sr matmuls, keeping it
            # off the critical path after the tile's last matmul
            nc.tensor.matmul(si[:], rxf_i[:, ms], fq[:, jr], start=True, stop=False)
            nc.tensor.matmul(si[:], rxf_nr[:, ms], fq[:, ji], start=False, stop=True)
            nc.tensor.matmul(sr[:], rxf_r[:, ms], fq[:, jr], start=True, stop=False)
            nc.tensor.matmul(sr[:], rxf_i[:, ms], fq[:, ji], start=False, stop=True)
            t2 = sq_pool.tile([128, NG], f32)
            nc.scalar.square(t2[:], si[:])
            nc.vector._custom_dve(
                sqadd, out=ob[:, j * NG : (j + 1) * NG], in0=sr[:], in1=t2[:]
            )

        def emit_out(m, gb, ob):
            ms = slice(m * 128, (m + 1) * 128)
            c0 = gb * 2 * NG
            if m == MB - 1 and gb == NGB - 1:
                # very last tile: split across both rings for a short drain
                nc.sync.dma_start(out[ms, c0 : c0 + NG], ob[:, 0:NG])
                nc.scalar.dma_start(out[ms, c0 + NG : c0 + 2 * NG], ob[:, NG : 2 * NG])
            else:
                eng = nc.sync if (gb * MB + m) % 2 == 0 else nc.scalar
                eng.dma_start(out[ms, c0 : c0 + 2 * NG], ob[:])

        # gb=0 with the DFT interleaved: chunk c unlocks m-blocks 2c, 2c+1
        obs = {}
        dft_chunk(0)
        for ci in range(nrc):
            ma, mb_ = 2 * ci, 2 * ci + 1
            for m in (ma, mb_):
                obs[m] = out_pool.tile([128, 2 * NG], f32, name=f"ob0_{m}", tag="ob")
                tile_mg(m, 0, 0, obs[m])
            for m in (ma, mb_):
                tile_mg(m, 0, 1, obs[m])
                emit_out(m, 0, obs[m])
            if ci + 1 < nrc:
                dft_chunk(ci + 1)

        for gb in range(1, NGB):
            for m in range(MB):
                ob = out_pool.tile([128, 2 * NG], f32, tag="ob")
                tile_mg(m, gb, 0, ob)
                tile_mg(m, gb, 1, ob)
                emit_out(m, gb, ob)

    nc.compile()
    return nc


def _host_prep(rx_real, rx_imag, freq_real, freq_imag, temperature, bpc=BPC, t=T):
    """Layout marshaling only: shard/transpose/cast inputs for the cores."""
    FG = 1024
    lk = np.outer(np.arange(L), np.arange(L)).astype(np.float64)
    w = np.exp(-2j * np.pi * lk / L) / np.sqrt(L)  # ortho DFT matrix (symmetric)
    # fold the temperature scale into the DFT matrix: sim scales by 1/temp
    w = w / np.sqrt(np.float64(np.asarray(temperature)))
    w_r = w.real.astype(np.float32).astype(_BF16)
    w_i = w.imag.astype(np.float32).astype(_BF16)
    # packed [wr | -wi | wi], 4KB-class rows
    wp = np.ascontiguousarray(np.concatenate([w_r, -w_i, w_i], axis=1))

    fqt_r = freq_real[:t].T.astype(_BF16)  # [L, T]
    fqt_i = freq_imag[:t].T.astype(_BF16)
    # packed freq: per group g of FG columns, [fr_g | fi_g] -> 4KB rows.
    # group 0 is quarter-interleaved [fr00|fi00|fr01|fi01] so it can ship
    # as two half-size DMAs that unlock the first output tiles sooner.
    fqp = np.empty((L, 2 * t), _BF16)
    for g in range(t // FG):
        fqp[:, 2 * g * FG : (2 * g + 1) * FG] = fqt_r[:, g * FG : (g + 1) * FG]
        fqp[:, (2 * g + 1) * FG : (2 * g + 2) * FG] = fqt_i[:, g * FG : (g + 1) * FG]
    h = FG // 2
    g0 = np.concatenate(
        [fqt_r[:, 0:h], fqt_i[:, 0:h], fqt_r[:, h:FG], fqt_i[:, h:FG]], axis=1
    )
    fqp[:, 0 : 2 * FG] = g0
    fqp = np.ascontiguousarray(fqp)

    rxt_r = np.asarray(rx_real, np.float32).T.astype(_BF16)  # [L, B]
    rxt_i = np.asarray(rx_imag, np.float32).T.astype(_BF16)

    RC = 256  # must match kernel: per-chunk [rxr_c | rxi_c]
    in_maps = []
    for c in range(NCORES):
        cs = slice(c * bpc, (c + 1) * bpc)
        rr, ri = rxt_r[:, cs], rxt_i[:, cs]
        rxp = np.empty((L, 2 * bpc), _BF16)
        for k in range(bpc // RC):
            rxp[:, 2 * k * RC : (2 * k + 1) * RC] = rr[:, k * RC : (k + 1) * RC]
            rxp[:, (2 * k + 1) * RC : (2 * k + 2) * RC] = ri[:, k * RC : (k + 1) * RC]
        in_maps.append({"rxp": np.ascontiguousarray(rxp), "fqp": fqp, "wp": wp})
    return in_maps


def kernel(rx_real, rx_imag, freq_real, freq_imag, temperature):
    from concourse.bass_utils import run_bass_kernel_spmd

    if "nc" not in _CACHE:
        _CACHE["nc"] = build_nc()
    nc = _CACHE["nc"]

    in_maps = _host_prep(rx_real, rx_imag, freq_real, freq_imag, temperature)
    res = run_bass_kernel_spmd(nc, in_maps, core_ids=list(range(NCORES)))
    _CACHE["last_result"] = res
    return np.concatenate([r["out"] for r in res.results], axis=0)


# revision 25
# speedup vs baseline: 1.0155x; 1.0008x over previous
"""Trainium2 Bass kernel: batched complex-waveform similarity.

Math: reference computes
    bank = ifft_ortho(freq)                # [T, L] complex
    score = rx @ conj(bank).T              # [B, T] complex
    sim   = (score.re^2 + score.im^2) / temperature

Since the ortho DFT is unitary,  score = fft_ortho(rx) @ conj(freq).T.
So the kernel never builds the bank: it DFTs rx via a 128x128 bf16
matmul, then runs one big complex GEMM [B,L]x[L,T] in bf16 with fp32
PSUM accumulation, and a fused squared-magnitude epilogue.  The
1/temperature scale is folded into the DFT matrix host-side (score
scales by 1/sqrt(temp), sim by 1/temp), so the epilogue is exactly
sq(Sr) + Si^2 with no extra scale op.

Sharding: data-parallel over the rx batch dim across 8 NeuronCores;
freq (as a transposed bf16 [L, T] pair) is replicated on every core.

Per-core engine pipeline (main phase is PE-bound, ~216ns per matmul):
  PE   : DFT (bf16) + 512 bf16 matmuls [128,128]@[128,512] -> PSUM Sr/Si
  ACT  : t2 = Square(Si)                    (PSUM -> SBUF)
  DVE  : out = Sr^2 + t2                    (custom fused DVE op, 1cyc/elem)
  SP/ACT: HWDGE DMAs in/out on both rings
"""

import numpy as np
import ml_dtypes

B = 8192
T = 8192
L = 128
NCORES = 8
BPC = B // NCORES  # batch rows per core

_BF16 = ml_dtypes.bfloat16

_CACHE = {}


# --------------------------------------------------------------------------- #
# Custom DVE op: out = Src0^2 + Src1   (2 ALU ops -> 1 cycle/elem)
# (Src0 = Sr from PSUM, Src1 = Si^2 staged by ScalarE)
# --------------------------------------------------------------------------- #
def _get_sqadd_op():
    import concourse.dve_ops as dve_ops
    from concourse.dve_spec import Spec, Src0, Src1, sq, lower, _has_src1
    from concourse.dve_uop import DveOpSpec

    name = "SQ_ADD2_ANT"
    for op in dve_ops.OPS:
        if op.name == name:
            return op

    spec = Spec(
        body=sq(Src0) + Src1,
        reference=lambda in0, in1, s0, s1, imm2: (
            in0.astype(np.float32) ** 2 + in1.astype(np.float32)
        ).astype(np.float32),
    )
    opcode = dve_ops._CUSTOM_DVE_ROW_BASE + len(dve_ops.OPS)
    assert opcode < 0x20
    shas = {}
    for ver in ("v3", "v4"):
        compiled = DveOpSpec(
            name=name, opcode=opcode, uops=lower(spec, ver=ver), rd1_en=_has_src1(spec)
        )
        shas[ver] = compiled.sha(ver)
    op = dve_ops.DveOp(name, spec, subdim=False, uops_sha=shas)
    dve_ops.OPS.append(op)
    dve_ops.CUSTOM_DVE_SPECS[name] = spec
    dve_ops._SUB_OPCODE_FOR_NAME[name] = opcode
    return op


# --------------------------------------------------------------------------- #
# Bass program (one SPMD NeuronCore)
# --------------------------------------------------------------------------- #
def build_nc(bpc=BPC, t=T, debug=False):
    from contextlib import ExitStack

    import concourse.bacc as bacc
    import concourse.bass as bass
    import concourse.mybir as mybir
    import concourse.tile as tile

    f32 = mybir.dt.float32
    bf16 = mybir.dt.bfloat16
    sqadd = _get_sqadd_op()

    NG = 512   # output columns per PSUM group (1 bank)
    FG = 1024  # freq columns per SBUF tile / DMA
    assert bpc % 128 == 0 and t % FG == 0

    nc = bacc.Bacc("TRN2", target_bir_lowering=False, debug=debug, num_devices=NCORES)

    # packed inputs: 4KB DRAM rows -> full-size DMA packets
    rxp = nc.dram_tensor("rxp", [L, 2 * bpc], bf16, kind="ExternalInput")
    fqp = nc.dram_tensor("fqp", [L, 2 * t], bf16, kind="ExternalInput")
    wp = nc.dram_tensor("wp", [L, 3 * L], bf16, kind="ExternalInput")
    out = nc.dram_tensor("out", [bpc, t], f32, kind="ExternalOutput")

    with tile.TileContext(nc) as tc, ExitStack() as ctx:
        consts = ctx.enter_context(tc.tile_pool(name="consts", bufs=1))
        psum = ctx.enter_context(
            tc.tile_pool(name="psum", bufs=4, space=bass.MemorySpace.PSUM)
        )
        sq_pool = ctx.enter_context(tc.tile_pool(name="sq", bufs=6))
        out_pool = ctx.enter_context(tc.tile_pool(name="ob", bufs=14))

        # ---- input DMA triggers first ---------------------------------- #
        # Everything packed to 4KB DRAM rows.  rx (one 512KB DMA) on the SP
        # ring; W + freq group 0 lead the ScalarE ring; remaining freq
        # groups (512KB [fr_g|fi_g] pairs) alternate across both rings.
        # rx in 4 chunked DMAs alternating rings: chunk c holds
        # [rxr[:, c*256:(c+1)*256] | rxi[:, ...]] so DFT chunk c waits only
        # on its own 128KB transfer, not the whole 512KB.
        # Ring schedule (the first ~10us of DMA runs at ramp-limited rate, so
        # order by when the g-major main loop needs each piece):
        #   SP ring    : rx_c0, rx_c2, fq1, fq2, fq4, fq6
        #   ScalarE    : wp, fq0a, fq0b, rx_c1, rx_c3, fq3, fq5, fq7
        RC = 256  # rx cols per chunk (per r/i half)
        nrc = bpc // RC
        w_sb = consts.tile([L, 3 * L], bf16)
        nc.sync.dma_start(w_sb[:], wp[:, :])
        rx_ck = [
            consts.tile([L, 2 * RC], bf16, tag=f"rx{c}", name=f"rx{c}")
            for c in range(nrc)
        ]
        fq_sb = [
            consts.tile([L, 2 * FG], bf16, tag=f"fq{g}", name=f"fq{g}")
            for g in range(t // FG)
        ]
        # group 0 leads the ScalarE ring as two quarter-interleaved halves
        # (see _host_prep); wp + all rx chunks stream on the SP ring in
        # parallel, so tile(0,0)'s inputs race through both rings at once.
        nc.scalar.dma_start(fq_sb[0][:, 0:FG], fqp[:, 0:FG])
        for c in range(nrc):
            nc.sync.dma_start(rx_ck[c][:], rxp[:, c * 2 * RC : (c + 1) * 2 * RC])
        nc.scalar.dma_start(fq_sb[0][:, FG : 2 * FG], fqp[:, FG : 2 * FG])
        # remaining groups: deadline for fq_g is ~13.8us*g after main start,
        # so ring balance barely matters; odd->SP (after rx), even->ScalarE.
        for g in range(1, t // FG):
            gs = slice(g * 2 * FG, (g + 1) * 2 * FG)
            eng = nc.sync if g % 2 == 1 else nc.scalar
            eng.dma_start(fq_sb[g][:], fqp[:, gs])

        # ---- PE warmup -------------------------------------------------- #
        # Dependency-free matmuls ramp the HAM clock gate while rx loads.
        warm_w = consts.tile([128, 128], bf16)
        nc.gpsimd.memset(warm_w[:], 0)
        warm_ps = psum.tile([128, NG], mybir.dt.float32, tag="si")
        for _ in range(24):
            nc.tensor.matmul(warm_ps[:, 0:128], warm_w[:], warm_w[:], start=True, stop=True)

        # ---- DFT of rx (bf16): rxfT = W' @ rxT -------------------------- #
        # W' = ortho DFT matrix / sqrt(temp), symmetric, so PE lhsT is W'.
        # rxfT_r = Wr@rxT_r - Wi@rxT_i ; rxfT_i = Wr@rxT_i + Wi@rxT_r
        rxf_r = consts.tile([L, bpc], bf16)
        rxf_i = consts.tile([L, bpc], bf16)
        rxf_nr = consts.tile([L, bpc], bf16)  # -rxfT_r
        wr = slice(0, L)
        wni = slice(L, 2 * L)
        wi = slice(2 * L, 3 * L)
        MB = bpc // 128
        NGB = t // FG  # fq tiles; each covers 2 NG-wide output groups
        f32p = mybir.dt.float32

        def dft_chunk(c):
            # rxfT chunk c (rxf cols [c*RC,(c+1)*RC] = m-blocks 2c, 2c+1)
            rc = rx_ck[c]
            ks = slice(c * RC, (c + 1) * RC)
            pr = psum.tile([128, NG], f32p, tag="sr")
            nc.tensor.matmul(pr[:, 0:RC], w_sb[:, wr], rc[:, 0:RC], start=True, stop=False)
            nc.tensor.matmul(pr[:, 0:RC], w_sb[:, wni], rc[:, RC : 2 * RC], start=False, stop=True)
            pi = psum.tile([128, NG], f32p, tag="si")
            nc.tensor.matmul(pi[:, 0:RC], w_sb[:, wr], rc[:, RC : 2 * RC], start=True, stop=False)
            nc.tensor.matmul(pi[:, 0:RC], w_sb[:, wi], rc[:, 0:RC], start=False, stop=True)
            nc.vector.tensor_copy(rxf_r[:, ks], pr[:, 0:RC])
            nc.vector.tensor_copy(rxf_i[:, ks], pi[:, 0:RC])
            nc.vector.tensor_scalar_mul(rxf_nr[:, ks], pr[:, 0:RC], -1.0)

        # ---- main complex GEMM + fused |.|^2 epilogue ------------------- #
        # Sr = rxf_r.T @ fr + rxf_i.T @ fi ; Si = rxf_i.T @ fr - rxf_r.T @ fi
        # g-major order: each 512KB freq pair feeds 13.8us of PE work, so
        # the ramp-limited early DMA never stalls the PE after tile 0.
        def tile_mg(m, gb, j, ob):
            ms = slice(m * 128, (m + 1) * 128)
            fq = fq_sb[gb]
            if gb == 0:
                # group 0 is quarter-interleaved [fr00|fi00|fr01|fi01]
                jr = slice(j * 2 * NG, j * 2 * NG + NG)
                ji = slice(j * 2 * NG + NG, (j + 1) * 2 * NG)
            else:
                jr = slice(j * NG, (j + 1) * NG)
                ji = slice(FG + j * NG, FG + (j + 1) * NG)
            sr = psum.tile([128, NG], f32p, tag="sr")
            si = psum.tile([128, NG], f32p, tag="si")
            # si first: the ACT square overlaps the sr matmuls, keeping it
            # off the critical path after the tile's last matmul
            nc.tensor.matmul(si[:], rxf_i[:, ms], fq[:, jr], start=True, stop=False)
            nc.tensor.matmul(si[:], rxf_nr[:, ms], fq[:, ji], start=False, stop=True)
            nc.tensor.matmul(sr[:], rxf_r[:, ms], fq[:, jr], start=True, stop=False)
            nc.tensor.matmul(sr[:], rxf_i[:, ms], fq[:, ji], start=False, stop=True)
            t2 = sq_pool.tile([128, NG], f32)
            nc.scalar.square(t2[:], si[:])
            nc.vector._custom_dve(
                sqadd, out=ob[:, j * NG : (j + 1) * NG], in0=sr[:], in1=t2[:]
            )

        def emit_out(m, gb, ob):
            ms = slice(m * 128, (m + 1) * 128)
            c0 = gb * 2 * NG
            if m == MB - 1 and gb == NGB - 1:
                # very last tile: split across both rings for a short drain
                nc.sync.dma_start(out[ms, c0 : c0 + NG], ob[:, 0:NG])
                nc.scalar.dma_start(out[ms, c0 + NG : c0 + 2 * NG], ob[:, NG : 2 * NG])
            else:
                eng = nc.sync if (gb * MB + m) % 2 == 0 else nc.scalar
                eng.dma_start(out[ms, c0 : c0 + 2 * NG], ob[:])

        # all DFT chunks up front (rx chunks stream back-to-back on the SP
        # ring); interleaving them into the main loop serializes the PSUM
        # recycle against the epilogue DVE queue and stalls the PE.
        for c in range(nrc):
            dft_chunk(c)

        for gb in range(NGB):
            for m in range(MB):
                ob = out_pool.tile([128, 2 * NG], f32, tag="ob")
                tile_mg(m, gb, 0, ob)
                tile_mg(m, gb, 1, ob)
                emit_out(m, gb, ob)

    nc.compile()
    return nc


def _host_prep(rx_real, rx_imag, freq_real, freq_imag, temperature, bpc=BPC, t=T):
    """Layout marshaling only: shard/transpose/cast inputs for the cores."""
    FG = 1024
    lk = np.outer(np.arange(L), np.arange(L)).astype(np.float64)
    w = np.exp(-2j * np.pi * lk / L) / np.sqrt(L)  # ortho DFT matrix (symmetric)
    # fold the temperature scale into the DFT matrix: sim scales by 1/temp
    w = w / np.sqrt(np.float64(np.asarray(temperature)))
    w_r = w.real.astype(np.float32).astype(_BF16)
    w_i = w.imag.astype(np.float32).astype(_BF16)
    # packed [wr | -wi | wi], 4KB-class rows
    wp = np.ascontiguousarray(np.concatenate([w_r, -w_i, w_i], axis=1))

    fqt_r = freq_real[:t].T.astype(_BF16)  # [L, T]
    fqt_i = freq_imag[:t].T.astype(_BF16)
    # packed freq: per group g of FG columns, [fr_g | fi_g] -> 4KB rows.
    # group 0 is quarter-interleaved [fr00|fi00|fr01|fi01] so it can ship
    # as two half-size DMAs that unlock the first output tiles sooner.
    fqp = np.empty((L, 2 * t), _BF16)
    for g in range(t // FG):
        fqp[:, 2 * g * FG : (2 * g + 1) * FG] = fqt_r[:, g * FG : (g + 1) * FG]
        fqp[:, (2 * g + 1) * FG : (2 * g + 2) * FG] = fqt_i[:, g * FG : (g + 1) * FG]
    h = FG // 2
    g0 = np.concatenate(
        [fqt_r[:, 0:h], fqt_i[:, 0:h], fqt_r[:, h:FG], fqt_i[:, h:FG]], axis=1
    )
    fqp[:, 0 : 2 * FG] = g0
    fqp = np.ascontiguousarray(fqp)

    rxt_r = np.asarray(rx_real, np.float32).T.astype(_BF16)  # [L, B]
    rxt_i = np.asarray(rx_imag, np.float32).T.astype(_BF16)

    RC = 256  # must match kernel: per-chunk [rxr_c | rxi_c]
    in_maps = []
    for c in range(NCORES):
        cs = slice(c * bpc, (c + 1) * bpc)
        rr, ri = rxt_r[:, cs], rxt_i[:, cs]
        rxp = np.empty((L, 2 * bpc), _BF16)
        for k in range(bpc // RC):
            rxp[:, 2 * k * RC : (2 * k + 1) * RC] = rr[:, k * RC : (k + 1) * RC]
            rxp[:, (2 * k + 1) * RC : (2 * k + 2) * RC] = ri[:, k * RC : (k + 1) * RC]
        in_maps.append({"rxp": np.ascontiguousarray(rxp), "fqp": fqp, "wp": wp})
    return in_maps


def kernel(rx_real, rx_imag, freq_real, freq_imag, temperature):
    from concourse.bass_utils import run_bass_kernel_spmd

    if "nc" not in _CACHE:
        _CACHE["nc"] = build_nc()
    nc = _CACHE["nc"]

    in_maps = _host_prep(rx_real, rx_imag, freq_real, freq_imag, temperature)
    res = run_bass_kernel_spmd(nc, in_maps, core_ids=list(range(NCORES)))
    _CACHE["last_result"] = res
    return np.concatenate([r["out"] for r in res.results], axis=0)


# revision 26
# speedup vs baseline: 1.0211x; 1.0055x over previous
"""Trainium2 Bass kernel: batched complex-waveform similarity.

Math: reference computes
    bank = ifft_ortho(freq)                # [T, L] complex
    score = rx @ conj(bank).T              # [B, T] complex
    sim   = (score.re^2 + score.im^2) / temperature

Since the ortho DFT is unitary,  score = fft_ortho(rx) @ conj(freq).T.
So the kernel never builds the bank: it DFTs rx via a 128x128 bf16
matmul, then runs one big complex GEMM [B,L]x[L,T] in bf16 with fp32
PSUM accumulation, and a fused squared-magnitude epilogue.  The
1/temperature scale is folded into the DFT matrix host-side (score
scales by 1/sqrt(temp), sim by 1/temp), so the epilogue is exactly
sq(Sr) + Si^2 with no extra scale op.

Sharding: data-parallel over the rx batch dim across 8 NeuronCores;
freq (as a transposed bf16 [L, T] pair) is replicated on every core.

Per-core engine pipeline (main phase is PE-bound, ~216ns per matmul):
  PE   : DFT (bf16) + 512 bf16 matmuls [128,128]@[128,512] -> PSUM Sr/Si
  ACT  : t2 = Square(Si)                    (PSUM -> SBUF)
  DVE  : out = Sr^2 + t2                    (custom fused DVE op, 1cyc/elem)
  SP/ACT: HWDGE DMAs in/out on both rings
"""

import numpy as np
import ml_dtypes

B = 8192
T = 8192
L = 128
NCORES = 8
BPC = B // NCORES  # batch rows per core

_BF16 = ml_dtypes.bfloat16

_CACHE = {}


# --------------------------------------------------------------------------- #
# Custom DVE op: out = Src0^2 + Src1   (2 ALU ops -> 1 cycle/elem)
# (Src0 = Sr from PSUM, Src1 = Si^2 staged by ScalarE)
# --------------------------------------------------------------------------- #
def _get_sqadd_op():
    import concourse.dve_ops as dve_ops
    from concourse.dve_spec import Spec, Src0, Src1, sq, lower, _has_src1
    from concourse.dve_uop import DveOpSpec

    name = "SQ_ADD2_ANT"
    for op in dve_ops.OPS:
        if op.name == name:
            return op

    spec = Spec(
        body=sq(Src0) + Src1,
        reference=lambda in0, in1, s0, s1, imm2: (
            in0.astype(np.float32) ** 2 + in1.astype(np.float32)
        ).astype(np.float32),
    )
    opcode = dve_ops._CUSTOM_DVE_ROW_BASE + len(dve_ops.OPS)
    assert opcode < 0x20
    shas = {}
    for ver in ("v3", "v4"):
        compiled = DveOpSpec(
            name=name, opcode=opcode, uops=lower(spec, ver=ver), rd1_en=_has_src1(spec)
        )
        shas[ver] = compiled.sha(ver)
    op = dve_ops.DveOp(name, spec, subdim=False, uops_sha=shas)
    dve_ops.OPS.append(op)
    dve_ops.CUSTOM_DVE_SPECS[name] = spec
    dve_ops._SUB_OPCODE_FOR_NAME[name] = opcode
    return op


# --------------------------------------------------------------------------- #
# Bass program (one SPMD NeuronCore)
# --------------------------------------------------------------------------- #
def build_nc(bpc=BPC, t=T, debug=False):
    from contextlib import ExitStack

    import concourse.bacc as bacc
    import concourse.bass as bass
    import concourse.mybir as mybir
    import concourse.tile as tile

    f32 = mybir.dt.float32
    bf16 = mybir.dt.bfloat16
    sqadd = _get_sqadd_op()

    NG = 512   # output columns per PSUM group (1 bank)
    FG = 1024  # freq columns per SBUF tile / DMA
    assert bpc % 128 == 0 and t % FG == 0

    nc = bacc.Bacc("TRN2", target_bir_lowering=False, debug=debug, num_devices=NCORES)

    # packed inputs: 4KB DRAM rows -> full-size DMA packets
    rxp = nc.dram_tensor("rxp", [L, 2 * bpc], bf16, kind="ExternalInput")
    fqp = nc.dram_tensor("fqp", [L, 2 * t], bf16, kind="ExternalInput")
    wp = nc.dram_tensor("wp", [L, 3 * L], bf16, kind="ExternalInput")
    out = nc.dram_tensor("out", [bpc, t], f32, kind="ExternalOutput")

    with tile.TileContext(nc) as tc, ExitStack() as ctx:
        consts = ctx.enter_context(tc.tile_pool(name="consts", bufs=1))
        psum = ctx.enter_context(
            tc.tile_pool(name="psum", bufs=4, space=bass.MemorySpace.PSUM)
        )
        sq_pool = ctx.enter_context(tc.tile_pool(name="sq", bufs=6))
        out_pool = ctx.enter_context(tc.tile_pool(name="ob", bufs=14))

        # ---- input DMA triggers first ---------------------------------- #
        # Everything packed to 4KB DRAM rows.  rx (one 512KB DMA) on the SP
        # ring; W + freq group 0 lead the ScalarE ring; remaining freq
        # groups (512KB [fr_g|fi_g] pairs) alternate across both rings.
        # rx in 4 chunked DMAs alternating rings: chunk c holds
        # [rxr[:, c*256:(c+1)*256] | rxi[:, ...]] so DFT chunk c waits only
        # on its own 128KB transfer, not the whole 512KB.
        # Ring schedule (the first ~10us of DMA runs at ramp-limited rate, so
        # order by when the g-major main loop needs each piece):
        #   SP ring    : rx_c0, rx_c2, fq1, fq2, fq4, fq6
        #   ScalarE    : wp, fq0a, fq0b, rx_c1, rx_c3, fq3, fq5, fq7
        RC = 256  # rx cols per chunk (per r/i half)
        nrc = bpc // RC
        w_sb = consts.tile([L, 3 * L], bf16)
        nc.sync.dma_start(w_sb[:], wp[:, :])
        rx_ck = [
            consts.tile([L, 2 * RC], bf16, tag=f"rx{c}", name=f"rx{c}")
            for c in range(nrc)
        ]
        fq_sb = [
            consts.tile([L, 2 * FG], bf16, tag=f"fq{g}", name=f"fq{g}")
            for g in range(t // FG)
        ]
        # group 0 leads the ScalarE ring as two quarter-interleaved halves
        # (see _host_prep); wp + all rx chunks stream on the SP ring in
        # parallel, so tile(0,0)'s inputs race through both rings at once.
        nc.scalar.dma_start(fq_sb[0][:, 0:FG], fqp[:, 0:FG])
        for c in range(nrc):
            nc.sync.dma_start(rx_ck[c][:], rxp[:, c * 2 * RC : (c + 1) * 2 * RC])
        nc.scalar.dma_start(fq_sb[0][:, FG : 2 * FG], fqp[:, FG : 2 * FG])
        # remaining groups: deadline for fq_g is ~13.8us*g after main start,
        # so ring balance barely matters; odd->SP (after rx), even->ScalarE.
        for g in range(1, t // FG):
            gs = slice(g * 2 * FG, (g + 1) * 2 * FG)
            eng = nc.sync if g % 2 == 1 else nc.scalar
            eng.dma_start(fq_sb[g][:], fqp[:, gs])

        # ---- PE warmup -------------------------------------------------- #
        # Dependency-free matmuls ramp the HAM clock gate while rx loads.
        warm_w = consts.tile([128, 128], bf16)
        nc.gpsimd.memset(warm_w[:], 0)
        warm_ps = psum.tile([128, NG], mybir.dt.float32, tag="si")
        for _ in range(24):
            nc.tensor.matmul(warm_ps[:, 0:128], warm_w[:], warm_w[:], start=True, stop=True)

        # ---- DFT of rx (bf16): rxfT = W' @ rxT -------------------------- #
        # W' = ortho DFT matrix / sqrt(temp), symmetric, so PE lhsT is W'.
        # rxfT_r = Wr@rxT_r - Wi@rxT_i ; rxfT_i = Wr@rxT_i + Wi@rxT_r
        rxf_r = consts.tile([L, bpc], bf16)
        rxf_i = consts.tile([L, bpc], bf16)
        rxf_nr = consts.tile([L, bpc], bf16)  # -rxfT_r
        wr = slice(0, L)
        wni = slice(L, 2 * L)
        wi = slice(2 * L, 3 * L)
        MB = bpc // 128
        NGB = t // FG  # fq tiles; each covers 2 NG-wide output groups
        f32p = mybir.dt.float32

        def dft_chunk(c):
            # rxfT chunk c (rxf cols [c*RC,(c+1)*RC] = m-blocks 2c, 2c+1)
            rc = rx_ck[c]
            ks = slice(c * RC, (c + 1) * RC)
            pr = psum.tile([128, NG], f32p, tag="sr")
            nc.tensor.matmul(pr[:, 0:RC], w_sb[:, wr], rc[:, 0:RC], start=True, stop=False)
            nc.tensor.matmul(pr[:, 0:RC], w_sb[:, wni], rc[:, RC : 2 * RC], start=False, stop=True)
            pi = psum.tile([128, NG], f32p, tag="si")
            nc.tensor.matmul(pi[:, 0:RC], w_sb[:, wr], rc[:, RC : 2 * RC], start=True, stop=False)
            nc.tensor.matmul(pi[:, 0:RC], w_sb[:, wi], rc[:, 0:RC], start=False, stop=True)
            nc.vector.tensor_copy(rxf_r[:, ks], pr[:, 0:RC])
            nc.vector.tensor_copy(rxf_i[:, ks], pi[:, 0:RC])
            nc.vector.tensor_scalar_mul(rxf_nr[:, ks], pr[:, 0:RC], -1.0)

        # ---- main complex GEMM + fused |.|^2 epilogue ------------------- #
        # Sr = rxf_r.T @ fr + rxf_i.T @ fi ; Si = rxf_i.T @ fr - rxf_r.T @ fi
        # g-major order: each 512KB freq pair feeds 13.8us of PE work, so
        # the ramp-limited early DMA never stalls the PE after tile 0.
        def tile_mg(m, gb, j, ob):
            ms = slice(m * 128, (m + 1) * 128)
            fq = fq_sb[gb]
            if gb == 0:
                # group 0 is quarter-interleaved [fr00|fi00|fr01|fi01]
                jr = slice(j * 2 * NG, j * 2 * NG + NG)
                ji = slice(j * 2 * NG + NG, (j + 1) * 2 * NG)
            else:
                jr = slice(j * NG, (j + 1) * NG)
                ji = slice(FG + j * NG, FG + (j + 1) * NG)
            sr = psum.tile([128, NG], f32p, tag="sr")
            si = psum.tile([128, NG], f32p, tag="si")
            # si first: the ACT square overlaps the sr matmuls, keeping it
            # off the critical path after the tile's last matmul
            nc.tensor.matmul(si[:], rxf_i[:, ms], fq[:, jr], start=True, stop=False)
            nc.tensor.matmul(si[:], rxf_nr[:, ms], fq[:, ji], start=False, stop=True)
            nc.tensor.matmul(sr[:], rxf_r[:, ms], fq[:, jr], start=True, stop=False)
            nc.tensor.matmul(sr[:], rxf_i[:, ms], fq[:, ji], start=False, stop=True)
            t2 = sq_pool.tile([128, NG], f32)
            nc.scalar.square(t2[:], si[:])
            nc.vector._custom_dve(
                sqadd, out=ob[:, j * NG : (j + 1) * NG], in0=sr[:], in1=t2[:]
            )

        # all DFT chunks up front (rx chunks stream back-to-back on the SP
        # ring); they finish ~2us before the first freq tile lands, so the
        # DFT is fully hidden under the input-DMA ramp.
        for c in range(nrc):
            dft_chunk(c)

        # m-major: the m=0 sweep consumes freq group gb at ~1.73us*(gb+1)
        # after main start, which the ramping input stream stays ahead of.
        for m in range(MB):
            ms = slice(m * 128, (m + 1) * 128)
            last_m = m == MB - 1
            for gb in range(NGB):
                c0 = gb * 2 * NG
                ob = out_pool.tile([128, 2 * NG], f32, tag="ob")
                tile_mg(m, gb, 0, ob)
                if last_m:
                    # final row: emit each half as soon as its DVE is done,
                    # on alternating rings, so the exit barrier waits least
                    nc.sync.dma_start(out[ms, c0 : c0 + NG], ob[:, 0:NG])
                tile_mg(m, gb, 1, ob)
                if last_m:
                    if gb == NGB - 1:
                        h = NG // 2
                        nc.sync.dma_start(
                            out[ms, c0 + NG : c0 + NG + h], ob[:, NG : NG + h]
                        )
                        nc.scalar.dma_start(
                            out[ms, c0 + NG + h : c0 + 2 * NG], ob[:, NG + h : 2 * NG]
                        )
                    else:
                        nc.scalar.dma_start(
                            out[ms, c0 + NG : c0 + 2 * NG], ob[:, NG : 2 * NG]
                        )
                else:
                    eng = nc.sync if (m * NGB + gb) % 2 == 0 else nc.scalar
                    eng.dma_start(out[ms, c0 : c0 + 2 * NG], ob[:])

    nc.compile()
    return nc


def _host_prep(rx_real, rx_imag, freq_real, freq_imag, temperature, bpc=BPC, t=T):
    """Layout marshaling only: shard/transpose/cast inputs for the cores."""
    FG = 1024
    lk = np.outer(np.arange(L), np.arange(L)).astype(np.float64)
    w = np.exp(-2j * np.pi * lk / L) / np.sqrt(L)  # ortho DFT matrix (symmetric)
    # fold the temperature scale into the DFT matrix: sim scales by 1/temp
    w = w / np.sqrt(np.float64(np.asarray(temperature)))
    w_r = w.real.astype(np.float32).astype(_BF16)
    w_i = w.imag.astype(np.float32).astype(_BF16)
    # packed [wr | -wi | wi], 4KB-class rows
    wp = np.ascontiguousarray(np.concatenate([w_r, -w_i, w_i], axis=1))

    fqt_r = freq_real[:t].T.astype(_BF16)  # [L, T]
    fqt_i = freq_imag[:t].T.astype(_BF16)
    # packed freq: per group g of FG columns, [fr_g | fi_g] -> 4KB rows.
    # group 0 is quarter-interleaved [fr00|fi00|fr01|fi01] so it can ship
    # as two half-size DMAs that unlock the first output tiles sooner.
    fqp = np.empty((L, 2 * t), _BF16)
    for g in range(t // FG):
        fqp[:, 2 * g * FG : (2 * g + 1) * FG] = fqt_r[:, g * FG : (g + 1) * FG]
        fqp[:, (2 * g + 1) * FG : (2 * g + 2) * FG] = fqt_i[:, g * FG : (g + 1) * FG]
    h = FG // 2
    g0 = np.concatenate(
        [fqt_r[:, 0:h], fqt_i[:, 0:h], fqt_r[:, h:FG], fqt_i[:, h:FG]], axis=1
    )
    fqp[:, 0 : 2 * FG] = g0
    fqp = np.ascontiguousarray(fqp)

    rxt_r = np.asarray(rx_real, np.float32).T.astype(_BF16)  # [L, B]
    rxt_i = np.asarray(rx_imag, np.float32).T.astype(_BF16)

    RC = 256  # must match kernel: per-chunk [rxr_c | rxi_c]
    in_maps = []
    for c in range(NCORES):
        cs = slice(c * bpc, (c + 1) * bpc)
        rr, ri = rxt_r[:, cs], rxt_i[:, cs]
        rxp = np.empty((L, 2 * bpc), _BF16)
        for k in range(bpc // RC):
            rxp[:, 2 * k * RC : (2 * k + 1) * RC] = rr[:, k * RC : (k + 1) * RC]
            rxp[:, (2 * k + 1) * RC : (2 * k + 2) * RC] = ri[:, k * RC : (k + 1) * RC]
        in_maps.append({"rxp": np.ascontiguousarray(rxp), "fqp": fqp, "wp": wp})
    return in_maps


def kernel(rx_real, rx_imag, freq_real, freq_imag, temperature):
    from concourse.bass_utils import run_bass_kernel_spmd

    if "nc" not in _CACHE:
        _CACHE["nc"] = build_nc()
    nc = _CACHE["nc"]

    in_maps = _host_prep(rx_real, rx_imag, freq_real, freq_imag, temperature)
    res = run_bass_kernel_spmd(nc, in_maps, core_ids=list(range(NCORES)))
    _CACHE["last_result"] = res
    return np.concatenate([r["out"] for r in res.results], axis=0)


# revision 30
# speedup vs baseline: 1.0287x; 1.0075x over previous
"""Trainium2 Bass kernel: batched complex-waveform similarity.

Math: reference computes
    bank = ifft_ortho(freq)                # [T, L] complex
    score = rx @ conj(bank).T              # [B, T] complex
    sim   = (score.re^2 + score.im^2) / temperature

Since the ortho DFT is unitary,  score = fft_ortho(rx) @ conj(freq).T.
So the kernel never builds the bank: it DFTs rx via a 128x128 bf16
matmul, then runs one big complex GEMM [B,L]x[L,T] in bf16 with fp32
PSUM accumulation, and a fused squared-magnitude epilogue.  The
1/temperature scale is folded into the DFT matrix host-side (score
scales by 1/sqrt(temp), sim by 1/temp), so the epilogue is exactly
sq(Sr) + Si^2 with no extra scale op.

Sharding: data-parallel over the rx batch dim across 8 NeuronCores;
freq (as a transposed bf16 [L, T] pair) is replicated on every core.

Per-core engine pipeline (main phase is PE-bound, ~216ns per matmul):
  PE   : DFT (bf16) + 512 bf16 matmuls [128,128]@[128,512] -> PSUM Sr/Si
  ACT  : t2 = Square(Si)                    (PSUM -> SBUF)
  DVE  : out = Sr^2 + t2                    (custom fused DVE op, 1cyc/elem)
  SP/ACT: HWDGE DMAs in/out on both rings
"""

import numpy as np
import ml_dtypes

B = 8192
T = 8192
L = 128
NCORES = 8
BPC = B // NCORES  # batch rows per core

_BF16 = ml_dtypes.bfloat16

_CACHE = {}


# --------------------------------------------------------------------------- #
# Custom DVE op: out = Src0^2 + Src1   (2 ALU ops -> 1 cycle/elem)
# (Src0 = Sr from PSUM, Src1 = Si^2 staged by ScalarE)
# --------------------------------------------------------------------------- #
def _get_sqadd_op():
    import concourse.dve_ops as dve_ops
    from concourse.dve_spec import Spec, Src0, Src1, sq, lower, _has_src1
    from concourse.dve_uop import DveOpSpec

    name = "SQ_ADD2_ANT"
    for op in dve_ops.OPS:
        if op.name == name:
            return op

    spec = Spec(
        body=sq(Src0) + Src1,
        reference=lambda in0, in1, s0, s1, imm2: (
            in0.astype(np.float32) ** 2 + in1.astype(np.float32)
        ).astype(np.float32),
    )
    opcode = dve_ops._CUSTOM_DVE_ROW_BASE + len(dve_ops.OPS)
    assert opcode < 0x20
    shas = {}
    for ver in ("v3", "v4"):
        compiled = DveOpSpec(
            name=name, opcode=opcode, uops=lower(spec, ver=ver), rd1_en=_has_src1(spec)
        )
        shas[ver] = compiled.sha(ver)
    op = dve_ops.DveOp(name, spec, subdim=False, uops_sha=shas)
    dve_ops.OPS.append(op)
    dve_ops.CUSTOM_DVE_SPECS[name] = spec
    dve_ops._SUB_OPCODE_FOR_NAME[name] = opcode
    return op


# --------------------------------------------------------------------------- #
# Bass program (one SPMD NeuronCore)
# --------------------------------------------------------------------------- #
def build_nc(bpc=BPC, t=T, debug=False):
    from contextlib import ExitStack

    import concourse.bacc as bacc
    import concourse.bass as bass
    import concourse.mybir as mybir
    import concourse.tile as tile

    f32 = mybir.dt.float32
    bf16 = mybir.dt.bfloat16
    sqadd = _get_sqadd_op()

    NG = 512   # output columns per PSUM group (1 bank)
    FG = 1024  # freq columns per SBUF tile / DMA
    assert bpc % 128 == 0 and t % FG == 0

    nc = bacc.Bacc("TRN2", target_bir_lowering=False, debug=debug, num_devices=NCORES)

    # packed inputs: 4KB DRAM rows -> full-size DMA packets
    rxp = nc.dram_tensor("rxp", [L, 2 * bpc], bf16, kind="ExternalInput")
    fqp = nc.dram_tensor("fqp", [L, 2 * t], bf16, kind="ExternalInput")
    wp = nc.dram_tensor("wp", [L, 3 * L], bf16, kind="ExternalInput")
    out = nc.dram_tensor("out", [bpc, t], f32, kind="ExternalOutput")

    with tile.TileContext(nc) as tc, ExitStack() as ctx:
        consts = ctx.enter_context(tc.tile_pool(name="consts", bufs=1))
        psum = ctx.enter_context(
            tc.tile_pool(name="psum", bufs=4, space=bass.MemorySpace.PSUM)
        )
        sq_pool = ctx.enter_context(tc.tile_pool(name="sq", bufs=6))
        out_pool = ctx.enter_context(tc.tile_pool(name="ob", bufs=14))

        # ---- input DMA triggers first ---------------------------------- #
        # Everything packed to 4KB DRAM rows.  rx (one 512KB DMA) on the SP
        # ring; W + freq group 0 lead the ScalarE ring; remaining freq
        # groups (512KB [fr_g|fi_g] pairs) alternate across both rings.
        # rx in 4 chunked DMAs alternating rings: chunk c holds
        # [rxr[:, c*256:(c+1)*256] | rxi[:, ...]] so DFT chunk c waits only
        # on its own 128KB transfer, not the whole 512KB.
        # Ring schedule (the first ~10us of DMA runs at ramp-limited rate, so
        # order by when the g-major main loop needs each piece):
        #   SP ring    : rx_c0, rx_c2, fq1, fq2, fq4, fq6
        #   ScalarE    : wp, fq0a, fq0b, rx_c1, rx_c3, fq3, fq5, fq7
        RC = 256  # rx cols per chunk (per r/i half)
        nrc = bpc // RC
        w_sb = consts.tile([L, 3 * L], bf16)
        nc.sync.dma_start(w_sb[:], wp[:, :])
        rx_ck = [
            consts.tile([L, 2 * RC], bf16, tag=f"rx{c}", name=f"rx{c}")
            for c in range(nrc)
        ]
        fq_sb = [
            consts.tile([L, 2 * FG], bf16, tag=f"fq{g}", name=f"fq{g}")
            for g in range(t // FG)
        ]
        # Startup critical path (early per-ring DMA rate is ~35-45 GB/s):
        # tile(0,0) needs wp + rx chunk 0 + fr00 + fi00 only.  Split those
        # across both rings; rx chunks 1-3 (first needed at main+27us) and
        # the later freq groups follow.
        # group 0 is quarter-interleaved [fr00|fi00|fr01|fi01] (_host_prep)
        nc.scalar.dma_start(fq_sb[0][:, NG : 2 * NG], fqp[:, NG : 2 * NG])  # fi00
        nc.sync.dma_start(rx_ck[0][:], rxp[:, 0 : 2 * RC])
        nc.sync.dma_start(fq_sb[0][:, 0:NG], fqp[:, 0:NG])  # fr00
        nc.scalar.dma_start(fq_sb[0][:, FG : 2 * FG], fqp[:, FG : 2 * FG])  # fq0b
        # m=0 consumes fq_g at ~1.73us*(g+1) after main start
        nc.sync.dma_start(fq_sb[1][:], fqp[:, 2 * FG : 4 * FG])
        nc.scalar.dma_start(fq_sb[2][:], fqp[:, 4 * FG : 6 * FG])
        nc.sync.dma_start(fq_sb[3][:], fqp[:, 6 * FG : 8 * FG])
        nc.scalar.dma_start(fq_sb[4][:], fqp[:, 8 * FG : 10 * FG])
        nc.sync.dma_start(fq_sb[5][:], fqp[:, 10 * FG : 12 * FG])
        nc.scalar.dma_start(fq_sb[6][:], fqp[:, 12 * FG : 14 * FG])
        nc.sync.dma_start(fq_sb[7][:], fqp[:, 14 * FG : 16 * FG])
        for c in range(1, nrc):
            nc.scalar.dma_start(rx_ck[c][:], rxp[:, c * 2 * RC : (c + 1) * 2 * RC])
        for g in range(8, t // FG):
            gs = slice(g * 2 * FG, (g + 1) * 2 * FG)
            eng = nc.sync if g % 2 == 0 else nc.scalar
            eng.dma_start(fq_sb[g][:], fqp[:, gs])

        # ---- PE warmup -------------------------------------------------- #
        # Dependency-free matmuls ramp the HAM clock gate while rx loads.
        warm_w = consts.tile([128, 128], bf16)
        nc.gpsimd.memset(warm_w[:], 0)
        warm_ps = psum.tile([128, NG], mybir.dt.float32, tag="si")
        for _ in range(24):
            nc.tensor.matmul(warm_ps[:, 0:128], warm_w[:], warm_w[:], start=True, stop=True)

        # ---- DFT of rx (bf16): rxfT = W' @ rxT -------------------------- #
        # W' = ortho DFT matrix / sqrt(temp), symmetric, so PE lhsT is W'.
        # rxfT_r = Wr@rxT_r - Wi@rxT_i ; rxfT_i = Wr@rxT_i + Wi@rxT_r
        rxf_r = consts.tile([L, bpc], bf16)
        rxf_i = consts.tile([L, bpc], bf16)
        rxf_nr = consts.tile([L, bpc], bf16)  # -rxfT_r
        wr = slice(0, L)
        wni = slice(L, 2 * L)
        wi = slice(2 * L, 3 * L)
        MB = bpc // 128
        NGB = t // FG  # fq tiles; each covers 2 NG-wide output groups
        f32p = mybir.dt.float32

        def dft_chunk(c):
            # rxfT chunk c (rxf cols [c*RC,(c+1)*RC] = m-blocks 2c, 2c+1)
            rc = rx_ck[c]
            ks = slice(c * RC, (c + 1) * RC)
            pr = psum.tile([128, NG], f32p, tag="sr")
            nc.tensor.matmul(pr[:, 0:RC], w_sb[:, wr], rc[:, 0:RC], start=True, stop=False)
            nc.tensor.matmul(pr[:, 0:RC], w_sb[:, wni], rc[:, RC : 2 * RC], start=False, stop=True)
            pi = psum.tile([128, NG], f32p, tag="si")
            nc.tensor.matmul(pi[:, 0:RC], w_sb[:, wr], rc[:, RC : 2 * RC], start=True, stop=False)
            nc.tensor.matmul(pi[:, 0:RC], w_sb[:, wi], rc[:, 0:RC], start=False, stop=True)
            nc.vector.tensor_copy(rxf_r[:, ks], pr[:, 0:RC])
            nc.vector.tensor_copy(rxf_i[:, ks], pi[:, 0:RC])
            nc.vector.tensor_scalar_mul(rxf_nr[:, ks], pr[:, 0:RC], -1.0)

        # ---- main complex GEMM + fused |.|^2 epilogue ------------------- #
        # Sr = rxf_r.T @ fr + rxf_i.T @ fi ; Si = rxf_i.T @ fr - rxf_r.T @ fi
        # g-major order: each 512KB freq pair feeds 13.8us of PE work, so
        # the ramp-limited early DMA never stalls the PE after tile 0.
        def tile_mg(m, gb, j, ob):
            ms = slice(m * 128, (m + 1) * 128)
            fq = fq_sb[gb]
            if gb == 0:
                # group 0 is quarter-interleaved [fr00|fi00|fr01|fi01]
                jr = slice(j * 2 * NG, j * 2 * NG + NG)
                ji = slice(j * 2 * NG + NG, (j + 1) * 2 * NG)
            else:
                jr = slice(j * NG, (j + 1) * NG)
                ji = slice(FG + j * NG, FG + (j + 1) * NG)
            sr = psum.tile([128, NG], f32p, tag="sr")
            si = psum.tile([128, NG], f32p, tag="si")
            # si completes first so the ACT square overlaps the sr matmuls;
            # lhsT order nr,i,i,r keeps the rxf_i LDWEIGHTS back-to-back
            nc.tensor.matmul(si[:], rxf_nr[:, ms], fq[:, ji], start=True, stop=False)
            nc.tensor.matmul(si[:], rxf_i[:, ms], fq[:, jr], start=False, stop=True)
            nc.tensor.matmul(sr[:], rxf_i[:, ms], fq[:, ji], start=True, stop=False)
            nc.tensor.matmul(sr[:], rxf_r[:, ms], fq[:, jr], start=False, stop=True)
            t2 = sq_pool.tile([128, NG], f32)
            nc.scalar.square(t2[:], si[:])
            nc.vector._custom_dve(
                sqadd, out=ob[:, j * NG : (j + 1) * NG], in0=sr[:], in1=t2[:]
            )

        # DFT chunk 0 unlocks the whole m=0 sweep (m-blocks 0,1 use rxf
        # chunk 0); chunks 1-3 are deferred to the m=0/m=1 boundary since
        # m=2 first needs them ~27us after main start.
        dft_chunk(0)

        # m-major: the m=0 sweep consumes freq group gb at ~1.73us*(gb+1)
        # after main start, which the ramping input stream stays ahead of.
        for m in range(MB):
            if m == 1:
                for c in range(1, nrc):
                    dft_chunk(c)
            ms = slice(m * 128, (m + 1) * 128)
            last_m = m == MB - 1
            for gb in range(NGB):
                c0 = gb * 2 * NG
                ob = out_pool.tile([128, 2 * NG], f32, tag="ob")
                tile_mg(m, gb, 0, ob)
                if last_m:
                    # final row: emit each half as soon as its DVE is done,
                    # on alternating rings, so the exit barrier waits least
                    nc.sync.dma_start(out[ms, c0 : c0 + NG], ob[:, 0:NG])
                tile_mg(m, gb, 1, ob)
                if last_m:
                    if gb == NGB - 1:
                        h = NG // 2
                        nc.sync.dma_start(
                            out[ms, c0 + NG : c0 + NG + h], ob[:, NG : NG + h]
                        )
                        nc.scalar.dma_start(
                            out[ms, c0 + NG + h : c0 + 2 * NG], ob[:, NG + h : 2 * NG]
                        )
                    else:
                        nc.scalar.dma_start(
                            out[ms, c0 + NG : c0 + 2 * NG], ob[:, NG : 2 * NG]
                        )
                else:
                    eng = nc.sync if (m * NGB + gb) % 2 == 0 else nc.scalar
                    eng.dma_start(out[ms, c0 : c0 + 2 * NG], ob[:])

    nc.compile()
    return nc


def _host_prep(rx_real, rx_imag, freq_real, freq_imag, temperature, bpc=BPC, t=T):
    """Layout marshaling only: shard/transpose/cast inputs for the cores."""
    FG = 1024
    lk = np.outer(np.arange(L), np.arange(L)).astype(np.float64)
    w = np.exp(-2j * np.pi * lk / L) / np.sqrt(L)  # ortho DFT matrix (symmetric)
    # fold the temperature scale into the DFT matrix: sim scales by 1/temp
    w = w / np.sqrt(np.float64(np.asarray(temperature)))
    w_r = w.real.astype(np.float32).astype(_BF16)
    w_i = w.imag.astype(np.float32).astype(_BF16)
    # packed [wr | -wi | wi], 4KB-class rows
    wp = np.ascontiguousarray(np.concatenate([w_r, -w_i, w_i], axis=1))

    fqt_r = freq_real[:t].T.astype(_BF16)  # [L, T]
    fqt_i = freq_imag[:t].T.astype(_BF16)
    # packed freq: per group g of FG columns, [fr_g | fi_g] -> 4KB rows.
    # group 0 is quarter-interleaved [fr00|fi00|fr01|fi01] so it can ship
    # as two half-size DMAs that unlock the first output tiles sooner.
    fqp = np.empty((L, 2 * t), _BF16)
    for g in range(t // FG):
        fqp[:, 2 * g * FG : (2 * g + 1) * FG] = fqt_r[:, g * FG : (g + 1) * FG]
        fqp[:, (2 * g + 1) * FG : (2 * g + 2) * FG] = fqt_i[:, g * FG : (g + 1) * FG]
    h = FG // 2
    g0 = np.concatenate(
        [fqt_r[:, 0:h], fqt_i[:, 0:h], fqt_r[:, h:FG], fqt_i[:, h:FG]], axis=1
    )
    fqp[:, 0 : 2 * FG] = g0
    fqp = np.ascontiguousarray(fqp)

    rxt_r = np.asarray(rx_real, np.float32).T.astype(_BF16)  # [L, B]
    rxt_i = np.asarray(rx_imag, np.float32).T.astype(_BF16)

    RC = 256  # must match kernel: per-chunk [rxr_c | rxi_c]
    in_maps = []
    for c in range(NCORES):
        cs = slice(c * bpc, (c + 1) * bpc)
        rr, ri = rxt_r[:, cs], rxt_i[:, cs]
        rxp = np.empty((L, 2 * bpc), _BF16)
        for k in range(bpc // RC):
            rxp[:, 2 * k * RC : (2 * k + 1) * RC] = rr[:, k * RC : (k + 1) * RC]
            rxp[:, (2 * k + 1) * RC : (2 * k + 2) * RC] = ri[:, k * RC : (k + 1) * RC]
        in_maps.append({"rxp": np.ascontiguousarray(rxp), "fqp": fqp, "wp": wp})
    return in_maps


def kernel(rx_real, rx_imag, freq_real, freq_imag, temperature):
    from concourse.bass_utils import run_bass_kernel_spmd

    if "nc" not in _CACHE:
        _CACHE["nc"] = build_nc()
    nc = _CACHE["nc"]

    in_maps = _host_prep(rx_real, rx_imag, freq_real, freq_imag, temperature)
    res = run_bass_kernel_spmd(nc, in_maps, core_ids=list(range(NCORES)))
    _CACHE["last_result"] = res
    return np.concatenate([r["out"] for r in res.results], axis=0)
